# revision 11
# baseline (speedup 1.0000x reference)
"""Trainium2 Bass kernel for strictly-causal RoPE self-attention (no softmax).

  out[b,h] = tril(rope(Q)@rope(Q)^T, -1) @ V    with K = Q.

Sharding: B*H = 8 independent (b,h) slices -> one per NeuronCore (pure data
parallel, no collectives). Per core: T=N=2048.

Per-core algorithm (device compute in bf16 matmul / f32 accumulate):
  - Host passes Q pre-transposed+deinterleaved (layout prep only):
      qte[n',t] = Q[t,2n'], qto[n',t] = Q[t,2n'+1]   [N/2, T]
    plus RoPE cos/sin tables in the same layout (input-independent constants;
    freqs are pair-constant so one table serves even+odd lanes).
  - Device RoPE (DVE, bf16 2x mode, full-width ops to amortize op overhead):
      qrt_e = qte*cos - qto*sin ; qrt_o = qto*cos + qte*sin
    giving QRT = rope(Q)^T as 16 [128, T] bf16 tiles (n on partitions).
    RoPE input DMAs are issued before V so the DVE starts ASAP.
  - Stage 1 (PE): P[s,t] = sum_n QRT[n,s]*QRT[n,t] for lower-triangle blocks
    in column-supersteps of 4 t-blocks. PSUM evictions on ScalarE (DVE stays
    dedicated to RoPE); the strict-causal mask of each diagonal 128x128 block
    is applied in-place by GpSimd.
  - Stage 2 (PE): out[t,n] += P[s,t]^T @ V[s,n], accumulating j in PSUM,
    evict on ScalarE, DMA out.
Superstep 0's stage-1 is emitted contraction-outer so the PE can chase the
RoPE stream tile-by-tile instead of waiting for all of it.
"""

import os
import sys
import math

for _p in ("/opt/trn_rl_repo", "/root/.axon_site/_ro/trn_rl_repo"):
    if os.path.isdir(_p) and _p not in sys.path:
        sys.path.append(_p)

import numpy as np
import ml_dtypes

B, H, T, N = 2, 4, 2048, 2048
THETA = 2.0 ** 16
NCORES = 8
CW = 512  # superstep width (t-columns) and output n-chunk width

bf16 = ml_dtypes.bfloat16

LAST_RESULT = None  # BassKernelResults of the most recent run (for test.py)


def build_bass(t_len=T, n_dim=N, num_devices=NCORES):
    from concourse import bacc, mybir, tile

    nc = bacc.Bacc("TRN2", target_bir_lowering=False, debug=False,
                   num_devices=num_devices)
    bf = mybir.dt.bfloat16
    f32 = mybir.dt.float32
    mult = mybir.AluOpType.mult

    nh = n_dim // 2
    kh = nh // 128           # n-tiles per half (8)
    kk_n = n_dim // 128      # total n-tiles (16)
    nb = t_len // 128        # t-blocks (16)
    ncks = t_len // CW       # supersteps (4)
    sw = CW // 128           # t-blocks per superstep (4)
    nch = n_dim // CW        # output n-chunks (4)

    qte = nc.declare_dram_parameter("qte", [nh, t_len], bf, isOutput=False)
    qto = nc.declare_dram_parameter("qto", [nh, t_len], bf, isOutput=False)
    cosd = nc.declare_dram_parameter("cosT", [nh, t_len], bf, isOutput=False)
    sind = nc.declare_dram_parameter("sinT", [nh, t_len], bf, isOutput=False)
    vin = nc.declare_dram_parameter("v", [t_len, n_dim], bf, isOutput=False)
    maskd = nc.declare_dram_parameter("mask", [128, 128], bf, isOutput=False)
    outd = nc.declare_dram_parameter("out", [t_len, n_dim], f32, isOutput=True)

    with tile.TileContext(nc) as tc:
        with (
            tc.tile_pool(name="qrt", bufs=kk_n) as qrt_pool,
            tc.tile_pool(name="vres", bufs=nb) as v_pool,
            tc.tile_pool(name="tbl", bufs=4) as tbl_pool,
            tc.tile_pool(name="rtmp", bufs=4) as tmp_pool,
            tc.tile_pool(name="ptile", bufs=20) as p_pool,
            tc.tile_pool(name="osb", bufs=6) as out_pool,
            tc.tile_pool(name="mk", bufs=1) as mk_pool,
            tc.tile_pool(name="psum", bufs=8, space="PSUM") as psum_pool,
        ):
            mask_sb = mk_pool.tile([128, 128], bf)
            nc.sync.dma_start(mask_sb[:], maskd[:])

            # RoPE first: its DMAs gate the whole pipeline.
            qrt = [None] * kk_n
            for kk in range(kh):
                te = qrt_pool.tile([128, t_len], bf, tag="qrt")
                to = qrt_pool.tile([128, t_len], bf, tag="qrt")
                ct = tbl_pool.tile([128, t_len], bf, tag="tbl")
                st = tbl_pool.tile([128, t_len], bf, tag="tbl")
                # partition-split loads: 4 DMAs per tile so the first tiles
                # spread across many queues and land ASAP
                for pr in range(0, 128, 32):
                    rr = slice(128 * kk + pr, 128 * kk + pr + 32)
                    pp = slice(pr, pr + 32)
                    nc.sync.dma_start(te[pp, :], qte[rr, :])
                    nc.sync.dma_start(to[pp, :], qto[rr, :])
                    nc.sync.dma_start(ct[pp, :], cosd[rr, :])
                    nc.sync.dma_start(st[pp, :], sind[rr, :])
                t_os = tmp_pool.tile([128, t_len], bf, tag="tmp")
                t_es = tmp_pool.tile([128, t_len], bf, tag="tmp")
                nc.vector.tensor_mul(t_os[:], to[:], st[:])   # O*S
                nc.vector.tensor_mul(t_es[:], te[:], st[:])   # E*S
                nc.vector.tensor_mul(te[:], te[:], ct[:])     # E*C
                nc.vector.tensor_sub(te[:], te[:], t_os[:])   # QRT_E
                nc.vector.tensor_mul(to[:], to[:], ct[:])     # O*C
                nc.vector.tensor_add(to[:], to[:], t_es[:])   # QRT_O
                qrt[kk] = te
                qrt[kh + kk] = to

            v_tiles = [None] * nb

            def load_v(jlo, jhi):
                for jb in range(jlo, min(jhi, nb)):
                    vt = v_pool.tile([128, n_dim], bf, tag="vt")
                    nc.sync.dma_start(vt[:], vin[128 * jb:128 * (jb + 1), :])
                    v_tiles[jb] = vt

            def evict_chain(ic, j, rj0, w, ps):
                pt = p_pool.tile([128, w], bf, tag="pt", name=f"pt_{ic}_{j}")
                nc.scalar.copy(pt[:, :], ps[:, :])
                if rj0 == 128 * j:
                    # diagonal block: strict-causal mask, off the DVE
                    nc.gpsimd.tensor_tensor(pt[:, 0:128], pt[:, 0:128],
                                            mask_sb[:], mult)
                return pt

            # Chase: stage-1 chains whose PSUM banks can all stay open
            # through the RoPE stream, emitted contraction-outer so the PE
            # consumes each RoPE tile as it lands. Superstep 0's chains plus
            # superstep 1's first `sw` chains = 8 open banks.
            chase_keys = [(0, j) for j in range(min(sw, nb))]
            if ncks > 1:
                chase_keys += [(1, j) for j in range(sw)]
            chase = []
            for ic, j in chase_keys:
                t0 = CW * ic
                rj0 = max(128 * j, t0)
                w = CW * (ic + 1) - rj0
                ps = psum_pool.tile([128, w], f32, tag="psum",
                                    name=f"ps_c{ic}_{j}")
                chase.append((ic, j, rj0, w, ps))
            for kk in range(kk_n):
                for ic, j, rj0, w, ps in chase:
                    nc.tensor.matmul(
                        ps[:, :], qrt[kk][:, 128 * j:128 * j + 128],
                        qrt[kk][:, rj0:rj0 + w],
                        start=(kk == 0), stop=(kk == kk_n - 1))
            chased = {}
            for ic, j, rj0, w, ps in chase:
                chased[(ic, j)] = (evict_chain(ic, j, rj0, w, ps), rj0)

            def stage1(ic):
                t0 = CW * ic
                ptiles = {}
                for j in range(sw * ic + sw):
                    if (ic, j) in chased:
                        ptiles[j] = chased[(ic, j)]
                        continue
                    rj0 = max(128 * j, t0)
                    w = CW * (ic + 1) - rj0
                    ps = psum_pool.tile([128, w], f32, tag="psum",
                                        name=f"ps_{ic}_{j}")
                    for kk in range(kk_n):
                        nc.tensor.matmul(
                            ps[:, :], qrt[kk][:, 128 * j:128 * j + 128],
                            qrt[kk][:, rj0:rj0 + w],
                            start=(kk == 0), stop=(kk == kk_n - 1))
                    ptiles[j] = (evict_chain(ic, j, rj0, w, ps), rj0)
                return ptiles

            def stage2(ic, ptiles):
                for d in range(sw):
                    i = sw * ic + d
                    ti = 128 * i
                    for ch in range(nch):
                        ops = psum_pool.tile([128, CW], f32, tag="psum",
                                             name=f"ps2_{i}_{ch}")
                        for j in range(i + 1):
                            pt, rj0 = ptiles[j]
                            off = ti - rj0
                            nc.tensor.matmul(
                                ops[:, :], pt[:, off:off + 128],
                                v_tiles[j][:, CW * ch:CW * (ch + 1)],
                                start=(j == 0), stop=(j == i))
                        osb = out_pool.tile([128, CW], f32, tag="osb",
                                            name=f"osb_{i}_{ch}")
                        nc.scalar.copy(osb[:], ops[:])
                        nc.sync.dma_start(
                            outd[ti:ti + 128, CW * ch:CW * (ch + 1)], osb[:])

            for c in range(ncks):
                load_v(sw * c, sw * (c + 1))
                ptiles = stage1(c)
                stage2(c, ptiles)

    nc.compile()
    return nc


def _tables(t_len=T, n_dim=N):
    t = np.arange(n_dim, dtype=np.float32)
    q = np.floor(t / 2.0) * 2.0
    f = (1.0 / THETA ** (q.astype(np.float64) / n_dim)
         / (2.0 * math.pi)).astype(np.float32)
    phases = np.arange(t_len, dtype=np.float32)[:, None] * f[None, :]
    ph = (phases % 1.0) * np.float32(2.0 * math.pi)
    ct = np.ascontiguousarray(np.cos(ph)[:, 0::2].T).astype(bf16)  # [N/2, T]
    st = np.ascontiguousarray(np.sin(ph)[:, 0::2].T).astype(bf16)
    return ct, st


def _mask128():
    s = np.arange(128)[:, None]
    tt = np.arange(128)[None, :]
    return (s < tt).astype(bf16)


_compiled = {}


def _get_nc():
    if "nc" not in _compiled:
        _compiled["nc"] = build_bass()
    return _compiled["nc"]


def kernel(Q, V):
    global LAST_RESULT
    from concourse.bass_utils import run_bass_kernel_spmd

    Q = np.asarray(Q)
    V = np.asarray(V)
    assert Q.shape == (B, H, T, N) and V.shape == (B, H, T, N)

    nc = _get_nc()
    ct, st = _tables()
    mask = _mask128()

    in_maps = []
    for b in range(B):
        for h in range(H):
            qs = Q[b, h]
            in_maps.append({
                "qte": np.ascontiguousarray(qs[:, 0::2].T).astype(bf16),
                "qto": np.ascontiguousarray(qs[:, 1::2].T).astype(bf16),
                "cosT": ct,
                "sinT": st,
                "v": V[b, h].astype(bf16),
                "mask": mask,
            })

    res = run_bass_kernel_spmd(nc, in_maps, core_ids=list(range(NCORES)))
    LAST_RESULT = res

    out = np.empty((B, H, T, N), dtype=np.float32)
    for b in range(B):
        for h in range(H):
            out[b, h] = res.results[b * H + h]["out"]
    return out
